# revision 2
# baseline (speedup 1.0000x reference)
"""Trainium2 Bass kernel for nn_BackgroundNoiseLayer.

Math: out[t, n*5+r] = sum_k spikes[t,k] * Wr[k, n*5+r]
  spikes (600,100) binary, from rest_of_brain < 0.25
  Wr (100, 200000) = scatter-add of edge values (host-side index preprocessing)

Distribution: 1D column-parallel over the 8 cores — each core gets a
25000-wide slab of Wr (its 5000 post-neurons x 5 receptors), spikes
replicated; per-core output slabs (600, 25000) are concatenated on host.

Device kernel (SPMD, identical on all cores): the harness gate is
rel_err < 2e-2, so the whole pipeline runs in bf16 (weights bf16, fp32
PSUM accumulate, bf16 output) — rel_err ~2e-3 while halving the HBM
traffic vs the fp32-exact variant (35 MB vs 70 MB per core). Per token
tile (128,128,128,128,88) stream matmuls (K=100, N=500, one PSUM bank)
into PSUM, copy+convert PSUM->bf16 SBUF staging alternating DVE/ACT, and
DMA 1.25 MB staging blocks to the DRAM output. Host upcasts to fp32.

Per-core traffic: 30 MB out write + 5 MB weight read + 0.12 MB spikes at
the ~358 GB/s per-NC HBM cap -> ~100 us floor; the fp32 baseline measured
205 us at 95% of the cap with exactly 2x the bytes.
"""

import numpy as np
import ml_dtypes

import concourse.bass as bass
import concourse.mybir as mybir
import concourse.tile as tile
from concourse.bass_utils import run_bass_kernel_spmd

BF16 = mybir.dt.bfloat16
F32 = mybir.dt.float32


# ---------------------------------------------------------------------------
# Workaround for walrus codegen limit on this toolchain: an instruction with
# more than one sync wait fails codegen ("Too many sync wait commands").
# Split every multi-wait instruction: extra waits move to single-wait NoOps
# inserted just before it on the same engine queue (same-engine FIFO dispatch
# preserves gating semantics).
# ---------------------------------------------------------------------------
def _split_multi_waits(nc):
    n_split = 0
    for fn in nc.m.functions:
        for bb in fn.blocks:
            new_list = []
            for inst in bb.instructions:
                si = inst.sync_info
                waits = list(si.on_wait) if si is not None and si.on_wait else []
                if len(waits) > 1:
                    for j, w in enumerate(waits[:-1]):
                        nop = mybir.InstNoOp(
                            name=f"{inst.name}_w{j}", ins=[], outs=[]
                        )
                        nop.engine = inst.engine
                        nop.sync_info = mybir.SyncInfo(on_wait=[w], on_update=[])
                        new_list.append(nop)
                        n_split += 1
                    inst.sync_info = mybir.SyncInfo(
                        on_wait=[waits[-1]], on_update=list(si.on_update or [])
                    )
                new_list.append(inst)
            bb.instructions = new_list
    return n_split


# ---------------------------------------------------------------------------
# Problem constants (hardcoded; kernel.py must be self-contained)
# ---------------------------------------------------------------------------
N_NEURONS = 40000
N_BKG = 100          # K (contraction dim)
N_SYN_BASIS = 5
T = 600              # BATCH * SEQ tokens
N_CORES = 8
NR = N_NEURONS * N_SYN_BASIS          # 200000 output columns
NR_CORE = NR // N_CORES               # 25000 per core

T_TILES = [128, 128, 128, 128, 88]    # sum = 600
CHUNK = 500                           # matmul N (psum bank limit: 512 fp32 out)
# Tapered staging groups: small first group lets the store stream start
# earlier, small last group shrinks the drain tail.
GROUPS = [2500, 5000, 5000, 5000, 5000, 2500]
GROUP_OFFS = [0, 2500, 7500, 12500, 17500, 22500, 25000]

_NC_CACHE = None


def _build_nc():
    nc = bass.Bass()
    spikes_t = nc.dram_tensor("spikes_t", [N_BKG, T], BF16, kind="ExternalInput")
    wr_hi = nc.dram_tensor("wr_hi", [N_BKG, NR_CORE], BF16, kind="ExternalInput")
    out = nc.dram_tensor("out", [T, NR_CORE], BF16, kind="ExternalOutput")

    with tile.TileContext(nc) as tc:
        with (
            tc.tile_pool(name="wpool", bufs=1) as wpool,
            tc.tile_pool(name="spool", bufs=1) as spool,
            tc.tile_pool(name="stage", bufs=3) as stage,
            tc.tile_pool(name="psum", bufs=8, space="PSUM") as psum,
        ):
            sp_sb = spool.tile([N_BKG, T], BF16)
            nc.sync.dma_start(sp_sb[:], spikes_t[:])
            w_sb = []                      # per-group SBUF weight tiles
            for g, gw in enumerate(GROUPS):
                gh = wpool.tile([N_BKG, gw], BF16, tag=f"wh{g}")
                sl = slice(GROUP_OFFS[g], GROUP_OFFS[g + 1])
                nc.sync.dma_start(gh[:], wr_hi[:, sl])
                w_sb.append(gh)

            copy_i = 0
            for ti, m in enumerate(T_TILES):
                t0 = ti * 128
                lhs = sp_sb[:, t0 : t0 + m]
                for g, gw in enumerate(GROUPS):
                    gh = w_sb[g]
                    st = stage.tile([m, gw], BF16, tag="st")
                    for c in range(gw // CHUNK):
                        cs = slice(c * CHUNK, (c + 1) * CHUNK)
                        ps = psum.tile([m, CHUNK], F32)
                        nc.tensor.matmul(
                            ps[:], lhs, gh[:, cs], start=True, stop=True
                        )
                        if copy_i % 2 == 0:
                            nc.vector.tensor_copy(st[:, cs], ps[:])
                        else:
                            nc.scalar.copy(st[:, cs], ps[:])
                        copy_i += 1
                    nc.scalar.dma_start(
                        out[t0 : t0 + m, GROUP_OFFS[g] : GROUP_OFFS[g + 1]],
                        st[:],
                    )
    _split_multi_waits(nc)
    return nc


def get_nc():
    global _NC_CACHE
    if _NC_CACHE is None:
        _NC_CACHE = _build_nc()
    return _NC_CACHE


def _host_preprocess(weights, synaptic_weights, rest_of_brain, post_idx, pre_idx,
                     syn_ids):
    spikes = (rest_of_brain.reshape(T, N_BKG) < 0.25).astype(np.float32)
    spikes_t = np.ascontiguousarray(spikes.T).astype(ml_dtypes.bfloat16)

    vals = weights[:, None] * synaptic_weights[syn_ids]            # (nnz, 5)
    cell = post_idx.astype(np.int64) * N_BKG + pre_idx.astype(np.int64)
    flat = (cell[:, None] * N_SYN_BASIS + np.arange(N_SYN_BASIS)[None, :]).ravel()
    w_dense = np.bincount(
        flat, weights=vals.astype(np.float64).ravel(),
        minlength=N_NEURONS * N_BKG * N_SYN_BASIS,
    ).astype(np.float32).reshape(N_NEURONS, N_BKG, N_SYN_BASIS)
    # Wr[k, n*5+r] = W[n, k, r]
    wr_full = np.ascontiguousarray(w_dense.transpose(1, 0, 2)).reshape(N_BKG, NR)
    wr_hi = wr_full.astype(ml_dtypes.bfloat16)
    return spikes_t, wr_hi


def kernel(**inputs) -> np.ndarray:
    weights = np.asarray(inputs["weights"], dtype=np.float32)
    synaptic_weights = np.asarray(inputs["synaptic_weights"], dtype=np.float32)
    rest_of_brain = np.asarray(inputs["rest_of_brain"], dtype=np.float32)
    post_idx = np.asarray(inputs["post_idx"])
    pre_idx = np.asarray(inputs["pre_idx"])
    syn_ids = np.asarray(inputs["syn_ids"])

    spikes_t, wr_hi = _host_preprocess(
        weights, synaptic_weights, rest_of_brain, post_idx, pre_idx, syn_ids
    )

    nc = get_nc()
    in_maps = [
        {
            "spikes_t": spikes_t,
            "wr_hi": np.ascontiguousarray(wr_hi[:, c * NR_CORE : (c + 1) * NR_CORE]),
        }
        for c in range(N_CORES)
    ]
    res = run_bass_kernel_spmd(nc, in_maps, core_ids=list(range(N_CORES)))
    out = np.concatenate(
        [res.results[c]["out"].astype(np.float32) for c in range(N_CORES)], axis=1
    )                                                              # (600, 200000)
    return out.reshape(1, T, NR)


# revision 8
# speedup vs baseline: 1.4903x; 1.4903x over previous
"""Trainium2 Bass kernel for nn_BackgroundNoiseLayer.

Math: out[t, n*5+r] = sum_k spikes[t,k] * Wr[k, n*5+r]
  spikes (600,100) binary, from rest_of_brain < 0.25
  Wr (100, 200000) = scatter-add of edge values (host-side index preprocessing)

Distribution: 1D column-parallel over the 8 cores — each core gets a
25000-wide slab of Wr (its 5000 post-neurons x 5 receptors), spikes
replicated; per-core output slabs (600, 25000) are concatenated on host.

Precision/traffic trade (harness gate: rel_err < 2e-2): the output is
stored as int8 with a per-column scale folded into the weights on the
host — W'[k,j] = Wr[k,j] * 126 / B[j] with B[j] = sum_k |Wr[k,j]| a hard
bound on |out[t,j]|, so the fp32 PSUM result is already in [-126,126]
and the PSUM->SBUF drain converts to int8 for free. The host multiplies
back by B[j]/126. Measured rel_err ~8e-3 (round) / ~1.5e-2 (truncate),
both under the gate. Per-core HBM traffic: 15 MB out + 5.1 MB weights +
0.12 MB spikes = 20.2 MB at the 360 GB/s per-NC cap -> ~56 us DMA floor
(vs 70 MB / 205 us for the fp32 baseline).

Device kernel (SPMD, identical on all cores): per token tile
(128,128,128,128,88) stream bf16 matmuls (K=100, N=500) into 2-bank PSUM
tiles; 1000-col strided copies drain PSUM->int8 SBUF staging, statically
load-balanced across DVE/ACT/Pool (three engines needed to keep up with
the DMA stream); all DMA (weight loads + tapered staging stores) is
issued from the otherwise-idle SP queue so copy-engine SEQs never block
on store semaphores.
"""

import numpy as np
import ml_dtypes

import concourse.bass as bass
import concourse.mybir as mybir
import concourse.tile as tile
from concourse.bass_utils import run_bass_kernel_spmd

BF16 = mybir.dt.bfloat16
F32 = mybir.dt.float32
I8 = mybir.dt.int8


# ---------------------------------------------------------------------------
# Workaround for walrus codegen limit on this toolchain: an instruction with
# more than one sync wait fails codegen ("Too many sync wait commands").
# Split every multi-wait instruction: extra waits move to single-wait NoOps
# inserted just before it on the same engine queue (same-engine FIFO dispatch
# preserves gating semantics).
# ---------------------------------------------------------------------------
def _split_multi_waits(nc):
    n_split = 0
    for fn in nc.m.functions:
        for bb in fn.blocks:
            new_list = []
            for inst in bb.instructions:
                si = inst.sync_info
                waits = list(si.on_wait) if si is not None and si.on_wait else []
                if len(waits) > 1:
                    for j, w in enumerate(waits[:-1]):
                        nop = mybir.InstNoOp(
                            name=f"{inst.name}_w{j}", ins=[], outs=[]
                        )
                        nop.engine = inst.engine
                        nop.sync_info = mybir.SyncInfo(on_wait=[w], on_update=[])
                        new_list.append(nop)
                        n_split += 1
                    inst.sync_info = mybir.SyncInfo(
                        on_wait=[waits[-1]], on_update=list(si.on_update or [])
                    )
                new_list.append(inst)
            bb.instructions = new_list
    return n_split


# ---------------------------------------------------------------------------
# Problem constants (hardcoded; kernel.py must be self-contained)
# ---------------------------------------------------------------------------
N_NEURONS = 40000
N_BKG = 100          # K (contraction dim)
N_SYN_BASIS = 5
T = 600              # BATCH * SEQ tokens
N_CORES = 8
NR = N_NEURONS * N_SYN_BASIS          # 200000 output columns
NR_CORE = NR // N_CORES               # 25000 per core

T_TILES = [128, 128, 128, 128, 88]    # sum = 600
CHUNK = 500                           # matmul N (one PSUM bank: 512 fp32)
BLK = 2 * CHUNK                       # cols per PSUM tile / per drain copy
# Store groups: large (>=8 blocks) so each ~3.6 us transfer hides the next
# store's ~1.3 us HWDGE dispatch on the single SP queue; small last group
# shortens the drain tail after the final copies.
GROUPS = [4000, 10000, 10000, 1000]
WSLAB = 5000                          # weight-load slab width

_NC_CACHE = None


def _build_nc():
    nc = bass.Bass()
    spikes_t = nc.dram_tensor("spikes_t", [N_BKG, T], BF16, kind="ExternalInput")
    wr = nc.dram_tensor("wr", [N_BKG, NR_CORE], BF16, kind="ExternalInput")
    out = nc.dram_tensor("out", [T, NR_CORE], I8, kind="ExternalOutput")

    with tile.TileContext(nc) as tc:
        with (
            tc.tile_pool(name="wpool", bufs=1) as wpool,
            tc.tile_pool(name="spool", bufs=1) as spool,
            tc.tile_pool(name="stage", bufs=6) as stage,
            tc.tile_pool(name="psum", bufs=4, space="PSUM") as psum,
        ):
            sp_sb = spool.tile([N_BKG, T], BF16)
            nc.sync.dma_start(sp_sb[:], spikes_t[:])
            w_sb = {}                      # weight slabs (5000 cols each)
            for wo in range(0, NR_CORE, WSLAB):
                gh = wpool.tile([N_BKG, WSLAB], BF16, tag=f"w{wo}")
                nc.sync.dma_start(gh[:], wr[:, wo : wo + WSLAB])
                w_sb[wo] = gh

            def wslice(c0, c1):
                base = (c0 // WSLAB) * WSLAB
                return w_sb[base][:, c0 - base : c1 - base]

            # PSUM can only be read by DVE and ACT (GPSIMD/Pool is rejected
            # by walrus: "GPSIMD Instructions cannot access PSUM"), so the
            # drain alternates the two in lockstep
            copy_rot = ["act", "dve"]
            copy_fns = {
                "dve": nc.vector.tensor_copy,
                "act": nc.scalar.copy,
            }
            copy_i = 0
            for ti, m in enumerate(T_TILES):
                t0 = ti * 128
                lhs = sp_sb[:, t0 : t0 + m]
                goff = 0
                for gw in GROUPS:
                    st = stage.tile([m, gw], I8, tag="st")
                    for b0 in range(0, gw, BLK):
                        ps = psum.tile([m, 2, 512], F32)
                        for h in range(2):
                            c0 = goff + b0 + h * CHUNK
                            nc.tensor.matmul(
                                ps[:, h, 0:CHUNK], lhs, wslice(c0, c0 + CHUNK),
                                start=True, stop=True,
                            )
                        eng = copy_rot[copy_i % 2]
                        copy_i += 1
                        copy_fns[eng](st[:, b0 : b0 + BLK], ps[:, 0:2, 0:CHUNK])
                    nc.sync.dma_start(out[t0 : t0 + m, goff : goff + gw], st[:])
                    goff += gw
    _split_multi_waits(nc)
    return nc


def get_nc():
    global _NC_CACHE
    if _NC_CACHE is None:
        _NC_CACHE = _build_nc()
    return _NC_CACHE


def _host_preprocess(weights, synaptic_weights, rest_of_brain, post_idx, pre_idx,
                     syn_ids):
    spikes = (rest_of_brain.reshape(T, N_BKG) < 0.25).astype(np.float32)
    spikes_t = np.ascontiguousarray(spikes.T).astype(ml_dtypes.bfloat16)

    vals = weights[:, None] * synaptic_weights[syn_ids]            # (nnz, 5)
    cell = post_idx.astype(np.int64) * N_BKG + pre_idx.astype(np.int64)
    flat = (cell[:, None] * N_SYN_BASIS + np.arange(N_SYN_BASIS)[None, :]).ravel()
    w_dense = np.bincount(
        flat, weights=vals.astype(np.float64).ravel(),
        minlength=N_NEURONS * N_BKG * N_SYN_BASIS,
    ).astype(np.float32).reshape(N_NEURONS, N_BKG, N_SYN_BASIS)
    # Wr[k, n*5+r] = W[n, k, r]
    wr_full = np.ascontiguousarray(w_dense.transpose(1, 0, 2)).reshape(N_BKG, NR)
    # Fold per-column int8 scales into the weights: B[j] bounds |out[:,j]|.
    col_bound = np.abs(wr_full).sum(axis=0)                        # (NR,)
    col_scale = 126.0 / np.maximum(col_bound, 1e-30)
    wr_scaled = (wr_full * col_scale[None, :]).astype(ml_dtypes.bfloat16)
    dequant = np.where(col_bound > 0, col_bound / 126.0, 0.0).astype(np.float32)
    return spikes_t, wr_scaled, dequant


def kernel(**inputs) -> np.ndarray:
    weights = np.asarray(inputs["weights"], dtype=np.float32)
    synaptic_weights = np.asarray(inputs["synaptic_weights"], dtype=np.float32)
    rest_of_brain = np.asarray(inputs["rest_of_brain"], dtype=np.float32)
    post_idx = np.asarray(inputs["post_idx"])
    pre_idx = np.asarray(inputs["pre_idx"])
    syn_ids = np.asarray(inputs["syn_ids"])

    spikes_t, wr_scaled, dequant = _host_preprocess(
        weights, synaptic_weights, rest_of_brain, post_idx, pre_idx, syn_ids
    )

    nc = get_nc()
    in_maps = [
        {
            "spikes_t": spikes_t,
            "wr": np.ascontiguousarray(wr_scaled[:, c * NR_CORE : (c + 1) * NR_CORE]),
        }
        for c in range(N_CORES)
    ]
    res = run_bass_kernel_spmd(nc, in_maps, core_ids=list(range(N_CORES)))
    q = np.concatenate(
        [res.results[c]["out"] for c in range(N_CORES)], axis=1
    )                                                              # (600, 200000) i8
    out = q.astype(np.float32) * dequant[None, :]
    return out.reshape(1, T, NR)


# revision 18
# speedup vs baseline: 1.5982x; 1.0723x over previous
"""Trainium2 Bass kernel for nn_BackgroundNoiseLayer.

Math: out[t, n*5+r] = sum_k spikes[t,k] * Wr[k, n*5+r]
  spikes (600,100) binary, from rest_of_brain < 0.25
  Wr (100, 200000) = scatter-add of edge values (host-side index preprocessing)

Distribution: 1D column-parallel over the 8 cores — each core gets a
25000-wide slab of Wr (its 5000 post-neurons x 5 receptors), spikes
replicated; per-core output slabs (600, 25000) are concatenated on host.

Precision/traffic trade (harness gate: rel_err < 2e-2): the output is
stored as int8 with a per-column scale folded into the weights on the
host — W'[k,j] = Wr[k,j] * 126 / B[j] with B[j] = sum_k |Wr[k,j]| a hard
bound on |out[t,j]|, so the fp32 PSUM result is already in [-126,126]
and the PSUM->SBUF drain converts to int8 for free. The host multiplies
back by B[j]/126. Measured rel_err ~8e-3 (round) / ~1.5e-2 (truncate),
both under the gate. Per-core HBM traffic: 15 MB out + 5.1 MB weights +
0.12 MB spikes = 20.2 MB at the 360 GB/s per-NC cap -> ~56 us DMA floor
(vs 70 MB / 205 us for the fp32 baseline).

Device kernel (SPMD, identical on all cores): per token tile
(128,128,128,128,88) stream bf16 matmuls (K=100, N=500) into 2-bank PSUM
tiles; 1000-col strided copies drain PSUM->int8 SBUF staging on DVE+ACT
(GPSIMD cannot read PSUM, so only these two engines can drain; they are
the bottleneck at ~68 us busy each with an 8:7 ACT:DVE interleave that
balances their 1018/1167 ns per-copy costs). The fp32->int8 convert in
the drain copy rounds to nearest on HW (measured rel_err 7.99e-3 equals
the host round model exactly). All DMA (tapered weight-slab loads +
tapered staging stores) is issued from the otherwise-idle SP queue.
"""

import numpy as np
import ml_dtypes

import concourse.bass as bass
import concourse.mybir as mybir
import concourse.tile as tile
from concourse.bass_utils import run_bass_kernel_spmd

BF16 = mybir.dt.bfloat16
F32 = mybir.dt.float32
I8 = mybir.dt.int8


# ---------------------------------------------------------------------------
# Workaround for walrus codegen limit on this toolchain: an instruction with
# more than one sync wait fails codegen ("Too many sync wait commands").
# Split every multi-wait instruction: extra waits move to single-wait NoOps
# inserted just before it on the same engine queue (same-engine FIFO dispatch
# preserves gating semantics).
# ---------------------------------------------------------------------------
def _split_multi_waits(nc):
    n_split = 0
    for fn in nc.m.functions:
        for bb in fn.blocks:
            new_list = []
            for inst in bb.instructions:
                si = inst.sync_info
                waits = list(si.on_wait) if si is not None and si.on_wait else []
                if len(waits) > 1:
                    for j, w in enumerate(waits[:-1]):
                        nop = mybir.InstNoOp(
                            name=f"{inst.name}_w{j}", ins=[], outs=[]
                        )
                        nop.engine = inst.engine
                        nop.sync_info = mybir.SyncInfo(on_wait=[w], on_update=[])
                        new_list.append(nop)
                        n_split += 1
                    inst.sync_info = mybir.SyncInfo(
                        on_wait=[waits[-1]], on_update=list(si.on_update or [])
                    )
                new_list.append(inst)
            bb.instructions = new_list
    return n_split


# ---------------------------------------------------------------------------
# Problem constants (hardcoded; kernel.py must be self-contained)
# ---------------------------------------------------------------------------
N_NEURONS = 40000
N_BKG = 100          # K (contraction dim)
N_SYN_BASIS = 5
T = 600              # BATCH * SEQ tokens
N_CORES = 8
NR = N_NEURONS * N_SYN_BASIS          # 200000 output columns
NR_CORE = NR // N_CORES               # 25000 per core

T_TILES = [128, 128, 128, 128, 88]    # sum = 600
CHUNK = 500                           # matmul N (one PSUM bank: 512 fp32)
BLK = 2 * CHUNK                       # cols per PSUM tile / per drain copy
# Store groups: large (>=8 blocks) so each ~3.6 us transfer hides the next
# store's ~1.3 us HWDGE dispatch on the single SP queue; small last group
# shortens the drain tail after the final copies.
GROUPS = [2000, 8000, 8000, 4000, 2000, 1000]
# Weight-load slabs: small first slab so the first matmul (and the drain
# pipeline behind it) starts ~2.5 us earlier; chunk boundaries (500) never
# straddle a slab edge.
WSLABS = [1000, 4000, 5000, 5000, 5000, 5000]
WSLAB_OFFS = [0, 1000, 5000, 10000, 15000, 20000, 25000]

_NC_CACHE = None


def _build_nc():
    nc = bass.Bass()
    spikes_t = nc.dram_tensor("spikes_t", [N_BKG, T], BF16, kind="ExternalInput")
    wr = nc.dram_tensor("wr", [N_BKG, NR_CORE], BF16, kind="ExternalInput")
    out = nc.dram_tensor("out", [T, NR_CORE], I8, kind="ExternalOutput")

    with tile.TileContext(nc) as tc:
        with (
            tc.tile_pool(name="wpool", bufs=1) as wpool,
            tc.tile_pool(name="spool", bufs=1) as spool,
            tc.tile_pool(name="stage", bufs=6) as stage,
            tc.tile_pool(name="psum", bufs=4, space="PSUM") as psum,
        ):
            sp_sb = spool.tile([N_BKG, T], BF16)
            nc.sync.dma_start(sp_sb[:], spikes_t[:])
            w_sb = []                      # weight slab tiles
            for si, sw in enumerate(WSLABS):
                gh = wpool.tile([N_BKG, sw], BF16, tag=f"w{si}")
                so = WSLAB_OFFS[si]
                nc.sync.dma_start(gh[:], wr[:, so : so + sw])
                w_sb.append(gh)

            def wslice(c0, c1):
                si = next(
                    i for i in range(len(WSLABS))
                    if WSLAB_OFFS[i] <= c0 and c1 <= WSLAB_OFFS[i + 1]
                )
                base = WSLAB_OFFS[si]
                return w_sb[si][:, c0 - base : c1 - base]

            # PSUM can only be read by DVE and ACT (GPSIMD/Pool is rejected
            # by walrus: "GPSIMD Instructions cannot access PSUM"). 8:7
            # interleave balances the per-copy costs (ACT 1018 ns vs DVE
            # 1167 ns) so both engines carry ~68 us.
            copy_rot = list("adadadadadadada")   # 8 ACT : 7 DVE per 15
            copy_fns = {
                "d": nc.vector.tensor_copy,
                "a": nc.scalar.copy,
            }
            copy_i = 0
            for ti, m in enumerate(T_TILES):
                t0 = ti * 128
                lhs = sp_sb[:, t0 : t0 + m]
                goff = 0
                for gw in GROUPS:
                    st = stage.tile([m, gw], I8, tag="st")
                    for b0 in range(0, gw, BLK):
                        ps = psum.tile([m, 2, 512], F32)
                        for h in range(2):
                            c0 = goff + b0 + h * CHUNK
                            nc.tensor.matmul(
                                ps[:, h, 0:CHUNK], lhs, wslice(c0, c0 + CHUNK),
                                start=True, stop=True,
                            )
                        eng = copy_rot[copy_i % len(copy_rot)]
                        copy_i += 1
                        copy_fns[eng](st[:, b0 : b0 + BLK], ps[:, 0:2, 0:CHUNK])
                    nc.sync.dma_start(out[t0 : t0 + m, goff : goff + gw], st[:])
                    goff += gw
    _split_multi_waits(nc)
    return nc


def get_nc():
    global _NC_CACHE
    if _NC_CACHE is None:
        _NC_CACHE = _build_nc()
    return _NC_CACHE


def _host_preprocess(weights, synaptic_weights, rest_of_brain, post_idx, pre_idx,
                     syn_ids):
    spikes = (rest_of_brain.reshape(T, N_BKG) < 0.25).astype(np.float32)
    spikes_t = np.ascontiguousarray(spikes.T).astype(ml_dtypes.bfloat16)

    vals = weights[:, None] * synaptic_weights[syn_ids]            # (nnz, 5)
    cell = post_idx.astype(np.int64) * N_BKG + pre_idx.astype(np.int64)
    flat = (cell[:, None] * N_SYN_BASIS + np.arange(N_SYN_BASIS)[None, :]).ravel()
    w_dense = np.bincount(
        flat, weights=vals.astype(np.float64).ravel(),
        minlength=N_NEURONS * N_BKG * N_SYN_BASIS,
    ).astype(np.float32).reshape(N_NEURONS, N_BKG, N_SYN_BASIS)
    # Wr[k, n*5+r] = W[n, k, r]
    wr_full = np.ascontiguousarray(w_dense.transpose(1, 0, 2)).reshape(N_BKG, NR)
    # Fold per-column int8 scales into the weights: B[j] bounds |out[:,j]|.
    col_bound = np.abs(wr_full).sum(axis=0)                        # (NR,)
    col_scale = 126.0 / np.maximum(col_bound, 1e-30)
    wr_scaled = (wr_full * col_scale[None, :]).astype(ml_dtypes.bfloat16)
    dequant = np.where(col_bound > 0, col_bound / 126.0, 0.0).astype(np.float32)
    return spikes_t, wr_scaled, dequant


def kernel(**inputs) -> np.ndarray:
    weights = np.asarray(inputs["weights"], dtype=np.float32)
    synaptic_weights = np.asarray(inputs["synaptic_weights"], dtype=np.float32)
    rest_of_brain = np.asarray(inputs["rest_of_brain"], dtype=np.float32)
    post_idx = np.asarray(inputs["post_idx"])
    pre_idx = np.asarray(inputs["pre_idx"])
    syn_ids = np.asarray(inputs["syn_ids"])

    spikes_t, wr_scaled, dequant = _host_preprocess(
        weights, synaptic_weights, rest_of_brain, post_idx, pre_idx, syn_ids
    )

    nc = get_nc()
    in_maps = [
        {
            "spikes_t": spikes_t,
            "wr": np.ascontiguousarray(wr_scaled[:, c * NR_CORE : (c + 1) * NR_CORE]),
        }
        for c in range(N_CORES)
    ]
    res = run_bass_kernel_spmd(nc, in_maps, core_ids=list(range(N_CORES)))
    q = np.concatenate(
        [res.results[c]["out"] for c in range(N_CORES)], axis=1
    )                                                              # (600, 200000) i8
    out = q.astype(np.float32) * dequant[None, :]
    return out.reshape(1, T, NR)


# revision 19
# speedup vs baseline: 1.6395x; 1.0259x over previous
"""Trainium2 Bass kernel for nn_BackgroundNoiseLayer.

Math: out[t, n*5+r] = sum_k spikes[t,k] * Wr[k, n*5+r]
  spikes (600,100) binary, from rest_of_brain < 0.25
  Wr (100, 200000) = scatter-add of edge values (host-side index preprocessing)

Distribution: 1D column-parallel over the 8 cores — each core gets a
25000-wide slab of Wr (its 5000 post-neurons x 5 receptors), spikes
replicated; per-core output slabs (600, 25000) are concatenated on host.

Precision/traffic trade (harness gate: rel_err < 2e-2): the output is
stored as int8 with a per-column scale folded into the weights on the
host — W'[k,j] = Wr[k,j] * 126 / B[j] with B[j] = sum_k |Wr[k,j]| a hard
bound on |out[t,j]|, so the fp32 PSUM result is already in [-126,126]
and the PSUM->SBUF drain converts to int8 for free. The host multiplies
back by B[j]/126. Measured rel_err ~8e-3 (round) / ~1.5e-2 (truncate),
both under the gate. Per-core HBM traffic: 15 MB out + 5.1 MB weights +
0.12 MB spikes = 20.2 MB at the 360 GB/s per-NC cap -> ~56 us DMA floor
(vs 70 MB / 205 us for the fp32 baseline).

Device kernel (SPMD, identical on all cores): per token tile
(128,128,128,128,88) stream bf16 matmuls (K=100, N=500) into 2-bank PSUM
tiles; 1000-col strided copies drain PSUM->int8 SBUF staging on DVE+ACT
(GPSIMD cannot read PSUM, so only these two engines can drain; they are
the bottleneck at ~68 us busy each with an 8:7 ACT:DVE interleave that
balances their 1018/1167 ns per-copy costs). The fp32->int8 convert in
the drain copy rounds to nearest on HW (measured rel_err 7.99e-3 equals
the host round model exactly). All DMA (tapered weight-slab loads +
tapered staging stores) is issued from the otherwise-idle SP queue.
"""

import numpy as np
import ml_dtypes

import concourse.bass as bass
import concourse.mybir as mybir
import concourse.tile as tile
from concourse.bass_utils import run_bass_kernel_spmd

BF16 = mybir.dt.bfloat16
F32 = mybir.dt.float32
I8 = mybir.dt.int8


# ---------------------------------------------------------------------------
# Workaround for walrus codegen limit on this toolchain: an instruction with
# more than one sync wait fails codegen ("Too many sync wait commands").
# Split every multi-wait instruction: extra waits move to single-wait NoOps
# inserted just before it on the same engine queue (same-engine FIFO dispatch
# preserves gating semantics).
# ---------------------------------------------------------------------------
def _split_multi_waits(nc):
    n_split = 0
    for fn in nc.m.functions:
        for bb in fn.blocks:
            new_list = []
            for inst in bb.instructions:
                si = inst.sync_info
                waits = list(si.on_wait) if si is not None and si.on_wait else []
                if len(waits) > 1:
                    for j, w in enumerate(waits[:-1]):
                        nop = mybir.InstNoOp(
                            name=f"{inst.name}_w{j}", ins=[], outs=[]
                        )
                        nop.engine = inst.engine
                        nop.sync_info = mybir.SyncInfo(on_wait=[w], on_update=[])
                        new_list.append(nop)
                        n_split += 1
                    inst.sync_info = mybir.SyncInfo(
                        on_wait=[waits[-1]], on_update=list(si.on_update or [])
                    )
                new_list.append(inst)
            bb.instructions = new_list
    return n_split


# ---------------------------------------------------------------------------
# Problem constants (hardcoded; kernel.py must be self-contained)
# ---------------------------------------------------------------------------
N_NEURONS = 40000
N_BKG = 100          # K (contraction dim)
N_SYN_BASIS = 5
T = 600              # BATCH * SEQ tokens
N_CORES = 8
NR = N_NEURONS * N_SYN_BASIS          # 200000 output columns
NR_CORE = NR // N_CORES               # 25000 per core

T_TILES = [128, 128, 128, 128, 88]    # sum = 600
CHUNK = 500                           # matmul N (one PSUM bank: 512 fp32)
BLK = 2 * CHUNK                       # cols per PSUM tile / per drain copy
# Column groups, tapered. The loop nest is COLUMN-MAJOR (outer over column
# groups, inner over token tiles): each weight slab is consumed 5x by the
# drain (~13.6 us per 5000-col group) while the next slab loads in 2.8 us,
# so weight loads always run ahead and the drain never starves (row-major
# chased the load stream at ~1795 col/us and idled ~3 us in lumps).
GROUPS = [1000, 4000, 5000, 5000, 5000, 4000, 1000]

_NC_CACHE = None


def _build_nc():
    nc = bass.Bass()
    spikes_t = nc.dram_tensor("spikes_t", [N_BKG, T], BF16, kind="ExternalInput")
    wr = nc.dram_tensor("wr", [N_BKG, NR_CORE], BF16, kind="ExternalInput")
    out = nc.dram_tensor("out", [T, NR_CORE], I8, kind="ExternalOutput")

    goffs = [0]
    for gw in GROUPS:
        goffs.append(goffs[-1] + gw)

    with tile.TileContext(nc) as tc:
        with (
            tc.tile_pool(name="wpool", bufs=1) as wpool,
            tc.tile_pool(name="spool", bufs=1) as spool,
            tc.tile_pool(name="stage", bufs=12) as stage,
            tc.tile_pool(name="psum", bufs=4, space="PSUM") as psum,
        ):
            sp_sb = spool.tile([N_BKG, T], BF16)
            nc.sync.dma_start(sp_sb[:], spikes_t[:])
            w_sb = []                      # one weight slab per column group
            for gi, gw in enumerate(GROUPS):
                gh = wpool.tile([N_BKG, gw], BF16, tag=f"w{gi}")
                nc.sync.dma_start(gh[:], wr[:, goffs[gi] : goffs[gi] + gw])
                w_sb.append(gh)

            # PSUM can only be read by DVE and ACT (GPSIMD/Pool is rejected
            # by walrus: "GPSIMD Instructions cannot access PSUM"). 8:7
            # interleave balances the per-copy costs (ACT 1018 ns vs DVE
            # 1167 ns) so both engines carry ~68 us.
            copy_rot = list("adadadaadadadad")   # 8 ACT : 7 DVE per 15
            copy_fns = {
                "d": nc.vector.tensor_copy,
                "a": nc.scalar.copy,
            }
            copy_i = 0
            for gi, gw in enumerate(GROUPS):
                goff = goffs[gi]
                for ti, m in enumerate(T_TILES):
                    t0 = ti * 128
                    lhs = sp_sb[:, t0 : t0 + m]
                    st = stage.tile([m, gw], I8, tag="st")
                    for b0 in range(0, gw, BLK):
                        ps = psum.tile([m, 2, 512], F32)
                        for h in range(2):
                            c0 = b0 + h * CHUNK
                            nc.tensor.matmul(
                                ps[:, h, 0:CHUNK], lhs,
                                w_sb[gi][:, c0 : c0 + CHUNK],
                                start=True, stop=True,
                            )
                        eng = copy_rot[copy_i % len(copy_rot)]
                        copy_i += 1
                        copy_fns[eng](st[:, b0 : b0 + BLK], ps[:, 0:2, 0:CHUNK])
                    nc.sync.dma_start(out[t0 : t0 + m, goff : goff + gw], st[:])
    _split_multi_waits(nc)
    return nc


def get_nc():
    global _NC_CACHE
    if _NC_CACHE is None:
        _NC_CACHE = _build_nc()
    return _NC_CACHE


def _host_preprocess(weights, synaptic_weights, rest_of_brain, post_idx, pre_idx,
                     syn_ids):
    spikes = (rest_of_brain.reshape(T, N_BKG) < 0.25).astype(np.float32)
    spikes_t = np.ascontiguousarray(spikes.T).astype(ml_dtypes.bfloat16)

    vals = weights[:, None] * synaptic_weights[syn_ids]            # (nnz, 5)
    cell = post_idx.astype(np.int64) * N_BKG + pre_idx.astype(np.int64)
    flat = (cell[:, None] * N_SYN_BASIS + np.arange(N_SYN_BASIS)[None, :]).ravel()
    w_dense = np.bincount(
        flat, weights=vals.astype(np.float64).ravel(),
        minlength=N_NEURONS * N_BKG * N_SYN_BASIS,
    ).astype(np.float32).reshape(N_NEURONS, N_BKG, N_SYN_BASIS)
    # Wr[k, n*5+r] = W[n, k, r]
    wr_full = np.ascontiguousarray(w_dense.transpose(1, 0, 2)).reshape(N_BKG, NR)
    # Fold per-column int8 scales into the weights: B[j] bounds |out[:,j]|.
    col_bound = np.abs(wr_full).sum(axis=0)                        # (NR,)
    col_scale = 126.0 / np.maximum(col_bound, 1e-30)
    wr_scaled = (wr_full * col_scale[None, :]).astype(ml_dtypes.bfloat16)
    dequant = np.where(col_bound > 0, col_bound / 126.0, 0.0).astype(np.float32)
    return spikes_t, wr_scaled, dequant


def kernel(**inputs) -> np.ndarray:
    weights = np.asarray(inputs["weights"], dtype=np.float32)
    synaptic_weights = np.asarray(inputs["synaptic_weights"], dtype=np.float32)
    rest_of_brain = np.asarray(inputs["rest_of_brain"], dtype=np.float32)
    post_idx = np.asarray(inputs["post_idx"])
    pre_idx = np.asarray(inputs["pre_idx"])
    syn_ids = np.asarray(inputs["syn_ids"])

    spikes_t, wr_scaled, dequant = _host_preprocess(
        weights, synaptic_weights, rest_of_brain, post_idx, pre_idx, syn_ids
    )

    nc = get_nc()
    in_maps = [
        {
            "spikes_t": spikes_t,
            "wr": np.ascontiguousarray(wr_scaled[:, c * NR_CORE : (c + 1) * NR_CORE]),
        }
        for c in range(N_CORES)
    ]
    res = run_bass_kernel_spmd(nc, in_maps, core_ids=list(range(N_CORES)))
    q = np.concatenate(
        [res.results[c]["out"] for c in range(N_CORES)], axis=1
    )                                                              # (600, 200000) i8
    out = q.astype(np.float32) * dequant[None, :]
    return out.reshape(1, T, NR)


# revision 21
# speedup vs baseline: 1.6573x; 1.0108x over previous
"""Trainium2 Bass kernel for nn_BackgroundNoiseLayer.

Math: out[t, n*5+r] = sum_k spikes[t,k] * Wr[k, n*5+r]
  spikes (600,100) binary, from rest_of_brain < 0.25
  Wr (100, 200000) = scatter-add of edge values (host-side index preprocessing)

Distribution: 1D column-parallel over the 8 cores — each core gets a
25000-wide slab of Wr (its 5000 post-neurons x 5 receptors), spikes
replicated; per-core output slabs (600, 25000) are concatenated on host.

Precision/traffic trade (harness gate: rel_err < 2e-2): the output is
stored as int8 with a per-column scale folded into the weights on the
host — W'[k,j] = Wr[k,j] * 126 / B[j] with B[j] = sum_k |Wr[k,j]| a hard
bound on |out[t,j]|, so the fp32 PSUM result is already in [-126,126]
and the PSUM->SBUF drain converts to int8 for free. The host multiplies
back by B[j]/126. Measured rel_err ~8e-3 (round) / ~1.5e-2 (truncate),
both under the gate. Per-core HBM traffic: 15 MB out + 5.1 MB weights +
0.12 MB spikes = 20.2 MB at the 360 GB/s per-NC cap -> ~56 us DMA floor
(vs 70 MB / 205 us for the fp32 baseline).

Device kernel (SPMD, identical on all cores): COLUMN-MAJOR loop nest —
outer over tapered column groups, inner over the 5 token tiles
(128,128,128,128,88) — so each weight slab is drained 5x (~13.6 us per
5000-col group) while the next slab loads in 2.8 us: weight loads always
run ahead and the drain never starves. bf16 matmuls (K=100, N=500)
stream into 2-bank PSUM tiles; 1000-col strided copies drain PSUM->int8
SBUF staging on DVE+ACT (GPSIMD cannot read PSUM, so only these two
engines can drain; they are the bottleneck at ~68 us busy each with an
8:7 ACT:DVE interleave balancing their 1018/1167 ns per-copy costs, both
>98% utilized in steady state). The fp32->int8 convert in the drain
rounds to nearest on HW (measured rel_err 7.99e-3 equals the host round
model exactly). All DMA is issued from the otherwise-idle SP queue;
deep (12-buf) int8 staging decouples stores from the drain.
"""

import numpy as np
import ml_dtypes

import concourse.bass as bass
import concourse.mybir as mybir
import concourse.tile as tile
from concourse.bass_utils import run_bass_kernel_spmd

BF16 = mybir.dt.bfloat16
F32 = mybir.dt.float32
I8 = mybir.dt.int8


# ---------------------------------------------------------------------------
# Workaround for walrus codegen limit on this toolchain: an instruction with
# more than one sync wait fails codegen ("Too many sync wait commands").
# Split every multi-wait instruction: extra waits move to single-wait NoOps
# inserted just before it on the same engine queue (same-engine FIFO dispatch
# preserves gating semantics).
# ---------------------------------------------------------------------------
def _split_multi_waits(nc):
    n_split = 0
    for fn in nc.m.functions:
        for bb in fn.blocks:
            new_list = []
            for inst in bb.instructions:
                si = inst.sync_info
                waits = list(si.on_wait) if si is not None and si.on_wait else []
                if len(waits) > 1:
                    for j, w in enumerate(waits[:-1]):
                        nop = mybir.InstNoOp(
                            name=f"{inst.name}_w{j}", ins=[], outs=[]
                        )
                        nop.engine = inst.engine
                        nop.sync_info = mybir.SyncInfo(on_wait=[w], on_update=[])
                        new_list.append(nop)
                        n_split += 1
                    inst.sync_info = mybir.SyncInfo(
                        on_wait=[waits[-1]], on_update=list(si.on_update or [])
                    )
                new_list.append(inst)
            bb.instructions = new_list
    return n_split


# ---------------------------------------------------------------------------
# Problem constants (hardcoded; kernel.py must be self-contained)
# ---------------------------------------------------------------------------
N_NEURONS = 40000
N_BKG = 100          # K (contraction dim)
N_SYN_BASIS = 5
T = 600              # BATCH * SEQ tokens
N_CORES = 8
NR = N_NEURONS * N_SYN_BASIS          # 200000 output columns
NR_CORE = NR // N_CORES               # 25000 per core

T_TILES = [128, 128, 128, 128, 88]    # sum = 600
CHUNK = 500                           # matmul N (one PSUM bank: 512 fp32)
BLK = 2 * CHUNK                       # cols per PSUM tile / per drain copy
# Column groups, tapered. The loop nest is COLUMN-MAJOR (outer over column
# groups, inner over token tiles): each weight slab is consumed 5x by the
# drain (~13.6 us per 5000-col group) while the next slab loads in 2.8 us,
# so weight loads always run ahead and the drain never starves (row-major
# chased the load stream at ~1795 col/us and idled ~3 us in lumps).
GROUPS = [1000, 4000, 5000, 5000, 5000, 3000, 2000]

_NC_CACHE = None


def _build_nc():
    nc = bass.Bass()
    spikes_t = nc.dram_tensor("spikes_t", [N_BKG, T], BF16, kind="ExternalInput")
    wr = nc.dram_tensor("wr", [N_BKG, NR_CORE], BF16, kind="ExternalInput")
    out = nc.dram_tensor("out", [T, NR_CORE], I8, kind="ExternalOutput")

    goffs = [0]
    for gw in GROUPS:
        goffs.append(goffs[-1] + gw)

    with tile.TileContext(nc) as tc:
        with (
            tc.tile_pool(name="wpool", bufs=1) as wpool,
            tc.tile_pool(name="spool", bufs=1) as spool,
            tc.tile_pool(name="stage", bufs=12) as stage,
            tc.tile_pool(name="psum", bufs=4, space="PSUM") as psum,
        ):
            sp_sb = spool.tile([N_BKG, T], BF16)
            nc.sync.dma_start(sp_sb[:], spikes_t[:])
            w_sb = []                      # one weight slab per column group
            for gi, gw in enumerate(GROUPS):
                gh = wpool.tile([N_BKG, gw], BF16, tag=f"w{gi}")
                nc.sync.dma_start(gh[:], wr[:, goffs[gi] : goffs[gi] + gw])
                w_sb.append(gh)

            # PSUM can only be read by DVE and ACT (GPSIMD/Pool is rejected
            # by walrus: "GPSIMD Instructions cannot access PSUM"). 8:7
            # interleave balances the per-copy costs (ACT 1018 ns vs DVE
            # 1167 ns) so both engines carry ~68 us.
            copy_rot = list("adadadaadadadad")   # 8 ACT : 7 DVE per 15
            copy_fns = {
                "d": nc.vector.tensor_copy,
                "a": nc.scalar.copy,
            }
            copy_i = 0
            for gi, gw in enumerate(GROUPS):
                goff = goffs[gi]
                for ti, m in enumerate(T_TILES):
                    t0 = ti * 128
                    lhs = sp_sb[:, t0 : t0 + m]
                    st = stage.tile([m, gw], I8, tag="st")
                    for b0 in range(0, gw, BLK):
                        ps = psum.tile([m, 2, 512], F32)
                        for h in range(2):
                            c0 = b0 + h * CHUNK
                            nc.tensor.matmul(
                                ps[:, h, 0:CHUNK], lhs,
                                w_sb[gi][:, c0 : c0 + CHUNK],
                                start=True, stop=True,
                            )
                        eng = copy_rot[copy_i % len(copy_rot)]
                        copy_i += 1
                        copy_fns[eng](st[:, b0 : b0 + BLK], ps[:, 0:2, 0:CHUNK])
                    nc.sync.dma_start(out[t0 : t0 + m, goff : goff + gw], st[:])
    _split_multi_waits(nc)
    return nc


def get_nc():
    global _NC_CACHE
    if _NC_CACHE is None:
        _NC_CACHE = _build_nc()
    return _NC_CACHE


def _host_preprocess(weights, synaptic_weights, rest_of_brain, post_idx, pre_idx,
                     syn_ids):
    spikes = (rest_of_brain.reshape(T, N_BKG) < 0.25).astype(np.float32)
    spikes_t = np.ascontiguousarray(spikes.T).astype(ml_dtypes.bfloat16)

    vals = weights[:, None] * synaptic_weights[syn_ids]            # (nnz, 5)
    cell = post_idx.astype(np.int64) * N_BKG + pre_idx.astype(np.int64)
    flat = (cell[:, None] * N_SYN_BASIS + np.arange(N_SYN_BASIS)[None, :]).ravel()
    w_dense = np.bincount(
        flat, weights=vals.astype(np.float64).ravel(),
        minlength=N_NEURONS * N_BKG * N_SYN_BASIS,
    ).astype(np.float32).reshape(N_NEURONS, N_BKG, N_SYN_BASIS)
    # Wr[k, n*5+r] = W[n, k, r]
    wr_full = np.ascontiguousarray(w_dense.transpose(1, 0, 2)).reshape(N_BKG, NR)
    # Fold per-column int8 scales into the weights: B[j] bounds |out[:,j]|.
    col_bound = np.abs(wr_full).sum(axis=0)                        # (NR,)
    col_scale = 126.0 / np.maximum(col_bound, 1e-30)
    wr_scaled = (wr_full * col_scale[None, :]).astype(ml_dtypes.bfloat16)
    dequant = np.where(col_bound > 0, col_bound / 126.0, 0.0).astype(np.float32)
    return spikes_t, wr_scaled, dequant


def kernel(**inputs) -> np.ndarray:
    weights = np.asarray(inputs["weights"], dtype=np.float32)
    synaptic_weights = np.asarray(inputs["synaptic_weights"], dtype=np.float32)
    rest_of_brain = np.asarray(inputs["rest_of_brain"], dtype=np.float32)
    post_idx = np.asarray(inputs["post_idx"])
    pre_idx = np.asarray(inputs["pre_idx"])
    syn_ids = np.asarray(inputs["syn_ids"])

    spikes_t, wr_scaled, dequant = _host_preprocess(
        weights, synaptic_weights, rest_of_brain, post_idx, pre_idx, syn_ids
    )

    nc = get_nc()
    in_maps = [
        {
            "spikes_t": spikes_t,
            "wr": np.ascontiguousarray(wr_scaled[:, c * NR_CORE : (c + 1) * NR_CORE]),
        }
        for c in range(N_CORES)
    ]
    res = run_bass_kernel_spmd(nc, in_maps, core_ids=list(range(N_CORES)))
    q = np.concatenate(
        [res.results[c]["out"] for c in range(N_CORES)], axis=1
    )                                                              # (600, 200000) i8
    out = q.astype(np.float32) * dequant[None, :]
    return out.reshape(1, T, NR)
